# revision 65
# baseline (speedup 1.0000x reference)
"""Multi-head attention block (B=16, N=577, C=1024, H=16) on 8 Trainium2 NeuronCores.

Sharding: data-parallel over batch — 2 batch elements per core, no collectives.

Device dataflow per batch element (fully "transposed" so no on-device transposes):
  inputs staged host-side: xT = x^T  [C,N] bf16, wqkvT = w_qkv^T [C,3C] bf16,
  wprojT = w_proj^T [C,C] bf16.
  qT,kT [o,n] <- (wqkvT tile).T @ xT      (o on partitions: per-head [64, n])
  V     [n,o] <- (xT tile).T @ wqkvT      (n on partitions: per-head [m, 64])
  S^T   [m,n] <- (kT_h tile [d,m]).T @ qT_h [d,n]          (d=64 contraction)
  P^T = exp(0.125 * S^T)                  (softmax numerator; max-subtraction skipped:
                                           scaled scores are ~N(0,1), |s|<~10, exp safe)
  OT'[0:64,n] = sum_m V_h[m,d] P^T[m,n];  OT'[64,n] = sum_m P^T[m,n]
      (one matmul: lhsT = [V_h | ones] [m, 65] — sumexp comes free as row 64)
  OT = OT'[0:64] * (1/OT'[64])            (softmax denominator)
  y[n,o] = (OT tile [c,n]).T @ wprojT + b_proj   (y stored bf16, widened on host)

Schedule: attention is software-pipelined one head-pair ahead — S/exp of pair
p+1 issue before PV of pair p, so the ACT-engine exp latency never gates the
PE.  QKV/V/proj matmuls of the other batch element fill the PE between S
chunk groups at roughly the ACT drain rate.
"""

import os
import sys

import numpy as np

if "/opt/trn_rl_repo" not in sys.path:
    sys.path.insert(0, "/opt/trn_rl_repo")

import ml_dtypes

B, N, C = 16, 577, 1024
H, D = 16, 64
P = 128
CT = C // P  # 8 contraction tiles
NT = 5  # n(row) tiles of 128: 4*128 + 65
NTS = [128, 128, 128, 128, 65]
NCH = [(0, 512), (512, 65)]  # free-dim chunks of 577 (psum bank = 512 fp32)
NCORES = 8
BPC = B // NCORES  # batches per core

_CACHE = {}
LAST_RESULT = None


def _ensure_ntff_hook():
    """Install antenv.axon_hooks with a ctypes-based NTFF profile hook if the
    environment's antenv package lacks it (mirrors trn_boot._ntff_profile_via_ctypes).
    Without this, run_bass_kernel_spmd(trace=True) silently skips tracing."""
    try:
        from antenv import axon_hooks  # noqa: F401

        return
    except ImportError:
        pass
    import contextlib
    import ctypes
    import types

    import antenv

    so_path = "/opt/axon/libaxon_pjrt.so"
    mod = types.ModuleType("antenv.axon_hooks")
    _state = {"hook": None, "set": False}

    def _make_hook():
        if not os.path.exists(so_path):
            return None
        lib = ctypes.CDLL(so_path)
        if not hasattr(lib, "axon_start_nrt_profile"):
            return None
        lib.axon_start_nrt_profile.argtypes = [
            ctypes.POINTER(ctypes.c_int64),
            ctypes.c_size_t,
        ]
        lib.axon_start_nrt_profile.restype = ctypes.c_int64
        lib.axon_stop_nrt_profile.argtypes = [ctypes.c_char_p]
        lib.axon_stop_nrt_profile.restype = ctypes.c_int64

        @contextlib.contextmanager
        def _hook(output_dir, device_ids):
            import jax

            jax.devices()
            if device_ids:
                ids = (ctypes.c_int64 * len(device_ids))(*device_ids)
                rc = lib.axon_start_nrt_profile(ids, len(device_ids))
            else:
                rc = lib.axon_start_nrt_profile(None, 0)
            if rc != 0:
                raise RuntimeError(f"axon_start_nrt_profile rc={rc}")
            try:
                yield
            finally:
                n = lib.axon_stop_nrt_profile(str(output_dir).encode())
                print(f"ntff profile: {n} file(s) written to {output_dir}", file=sys.stderr)

        return _hook

    def set_axon_ntff_profile_hook(h):
        _state["hook"] = h
        _state["set"] = True

    def get_axon_ntff_profile_hook():
        if not _state["set"]:
            set_axon_ntff_profile_hook(_make_hook())
        return _state["hook"]

    mod.set_axon_ntff_profile_hook = set_axon_ntff_profile_hook
    mod.get_axon_ntff_profile_hook = get_axon_ntff_profile_hook
    sys.modules["antenv.axon_hooks"] = mod
    antenv.axon_hooks = mod


def _build_nc():
    import concourse.bass as bass
    import concourse.tile as tile
    from concourse import bacc, mybir

    dtb = mybir.dt.bfloat16
    dtf = mybir.dt.float32
    Exp = mybir.ActivationFunctionType.Exp

    nc = bacc.Bacc(None, target_bir_lowering=False)

    xt = nc.dram_tensor("xt", [BPC, C, N], dtb, kind="ExternalInput")
    wq = nc.dram_tensor("wqkvT", [C, 3 * C], dtb, kind="ExternalInput")
    wp = nc.dram_tensor("wprojT", [C, C], dtb, kind="ExternalInput")
    bqk = nc.dram_tensor("bqk", [P, 16], dtf, kind="ExternalInput")
    bv = nc.dram_tensor("bv", [C], dtb, kind="ExternalInput")
    bpr = nc.dram_tensor("bproj", [C], dtb, kind="ExternalInput")
    y = nc.dram_tensor("y", [BPC, N, C], dtb, kind="ExternalOutput")

    from contextlib import ExitStack

    with tile.TileContext(nc) as tc:
        with ExitStack() as ctx:
            consts = ctx.enter_context(tc.tile_pool(name="consts", bufs=1))
            wpool = ctx.enter_context(tc.tile_pool(name="weights", bufs=1))
            xpool = ctx.enter_context(tc.tile_pool(name="xin", bufs=2))
            qkpool = ctx.enter_context(tc.tile_pool(name="qk", bufs=2))
            vpool = ctx.enter_context(tc.tile_pool(name="vv", bufs=2))
            epool = ctx.enter_context(tc.tile_pool(name="est", bufs=4))
            opool = ctx.enter_context(tc.tile_pool(name="ot", bufs=16))
            n2 = ctx.enter_context(tc.tile_pool(name="n2", bufs=4))
            n1 = ctx.enter_context(tc.tile_pool(name="n1", bufs=2))
            outpool = ctx.enter_context(tc.tile_pool(name="outs", bufs=2))
            psA = ctx.enter_context(tc.tile_pool(name="psA", bufs=3, space="PSUM"))
            psB = ctx.enter_context(tc.tile_pool(name="psB", bufs=2, space="PSUM"))

            wq_sb = wpool.tile([P, CT, 3 * C], dtb, tag="wq")
            wp_sb = wpool.tile([P, CT, C], dtb, tag="wp")
            bqk_sb = consts.tile([P, 16], dtf, tag="bqk")
            ones1 = consts.tile([1, P], dtb, tag="ones1")
            nc.vector.memset(ones1[:], 1.0)
            bvb_sb = consts.tile([P, C], dtb, tag="bvb")
            bpb_sb = consts.tile([P, C], dtb, tag="bpb")

            QS = [nc.sync, nc.gpsimd, nc.scalar]

            def load_x(b, qoff=0, split=False):
                x_sb = xpool.tile([P, CT, N], dtb, tag="x")
                xb = xt[b].rearrange("(ct p) n -> p ct n", p=P)
                if split:
                    # first-needed-first: cols 0:512 (first psum chunk) for
                    # every ct gate the first matmul; the 65-tail can trail.
                    for ct in range(CT):
                        QS[(ct + qoff) % 3].dma_start(
                            out=x_sb[:, ct, 0:512], in_=xb[:, ct, 0:512]
                        )
                    for ct in range(CT):
                        QS[(ct + qoff) % 3].dma_start(
                            out=x_sb[:, ct, 512:N], in_=xb[:, ct, 512:N]
                        )
                else:
                    for ct in range(CT):
                        QS[(ct + qoff) % 3].dma_start(out=x_sb[:, ct], in_=xb[:, ct])
                return x_sb

            def emit_qk_tile(x_sb, qk_sb, ot, wide):
                """qT/kT o-tile: psum [o,n] accumulated over ct, DVE evac+bias.
                wide=True uses one 2-bank psA tile + single evac; wide=False
                uses two 1-bank psB tiles (attention-phase filler)."""
                if wide:
                    ps = psA.tile([P, 640], dtf, tag="psA")
                    for (c0, cw) in NCH:
                        for ct in range(CT):
                            nc.tensor.matmul(
                                ps[:, c0:c0 + cw],
                                lhsT=wq_sb[:, ct, ot * P:(ot + 1) * P],
                                rhs=x_sb[:, ct, c0:c0 + cw],
                                start=(ct == 0),
                                stop=(ct == CT - 1),
                            )
                    nc.vector.tensor_scalar_add(
                        out=qk_sb[:, ot, :],
                        in0=ps[:, :N],
                        scalar1=bqk_sb[:, ot:ot + 1],
                    )
                    return
                for (c0, cw) in NCH:
                    ps = psB.tile([P, 512], dtf, tag="psB")
                    for ct in range(CT):
                        nc.tensor.matmul(
                            ps[:, :cw],
                            lhsT=wq_sb[:, ct, ot * P:(ot + 1) * P],
                            rhs=x_sb[:, ct, c0:c0 + cw],
                            start=(ct == 0),
                            stop=(ct == CT - 1),
                        )
                    nc.vector.tensor_scalar_add(
                        out=qk_sb[:, ot, c0:c0 + cw],
                        in0=ps[:, :cw],
                        scalar1=bqk_sb[:, ot:ot + 1],
                    )

            def emit_v_chunk(x_sb, v_sb, nt, oc):
                """V 512-col chunk: psum [n,o]; single strided evac+bias into
                the per-head 65-slot layout (col 64 of each slot is the ones
                column for the free sumexp row)."""
                nh = NTS[nt]
                ps = psB.tile([P, 512], dtf, tag="psB")
                for ct in range(CT):
                    nc.tensor.matmul(
                        ps[:nh],
                        lhsT=x_sb[:, ct, nt * P:nt * P + nh],
                        rhs=wq_sb[:, ct, 2 * C + oc * 512:2 * C + (oc + 1) * 512],
                        start=(ct == 0),
                        stop=(ct == CT - 1),
                    )
                v4 = v_sb[:].rearrange("p nt (h c) -> p nt h c", c=65)
                nc.vector.tensor_add(
                    out=v4[:nh, nt, oc * 8:(oc + 1) * 8, 0:64],
                    in0=ps[:nh].rearrange("p (h c) -> p h c", c=64),
                    in1=bvb_sb[:nh, oc * 512:(oc + 1) * 512].rearrange(
                        "p (h c) -> p h c", c=64
                    ),
                )

            def alloc_v(b):
                v_sb = vpool.tile([P, NT, H * 65], dtb, tag="v")
                v4 = v_sb[:].rearrange("p nt (h c) -> p nt h c", c=65)
                nc.vector.memset(v4[:, :, :, 64], 1.0)
                return v_sb

            def emit_S(qk_sb, hp, fill=()):
                """S^T + exp for head pair hp (both heads, all 5 m-tiles).
                `fill` thunks are interleaved after m-tiles 1 and 3 so the ACT
                engine (745ns/exp vs ~490ns of S matmuls per m-tile) can drain
                and release psA slots before the PE needs them.
                Returns (estA, estB) for the later PV."""
                ob = hp
                fill = list(fill)
                estA = epool.tile([P, NT, N], dtb, tag="est")
                estB = epool.tile([P, NT, N], dtb, tag="est")
                for mt in range(NT):
                    mh = NTS[mt]
                    psa = psA.tile([P, 640], dtf, tag="psA")
                    for (c0, cw) in NCH:
                        nc.tensor.matmul(
                            psa[:mh, c0:c0 + cw],
                            lhsT=qk_sb[0:64, 8 + ob, mt * P:mt * P + mh],
                            rhs=qk_sb[0:64, ob, c0:c0 + cw],
                        )
                    nc.scalar.activation(
                        out=estA[:mh, mt, :], in_=psa[:mh, :N], func=Exp, scale=0.125
                    )
                    psb = psA.tile([P, 640], dtf, tag="psA")
                    for (c0, cw) in NCH:
                        nc.tensor.matmul(
                            psb[:mh, c0:c0 + cw],
                            lhsT=qk_sb[64:128, 8 + ob, mt * P:mt * P + mh],
                            rhs=qk_sb[64:128, ob, c0:c0 + cw],
                        )
                    nc.scalar.activation(
                        out=estB[:mh, mt, :], in_=psb[:mh, :N], func=Exp, scale=0.125
                    )
                    if mt in (1, 3) and fill:
                        fill.pop(0)()
                return estA, estB

            def emit_PV(v_sb, ests, ot_sb, hp, act_evac=False):
                """PV(+sumexp) for head pair hp from prebuilt est tiles.
                m-tile 4 is zero-padded to K=128 (est/v guard rows are 0).
                act_evac routes otr to the ACT engine (idle at the kernel
                tail) so the reciprocal chain starts in parallel on DVE.
                Returns deferred normalize-mul thunks (all-bf16, DVE 2x)."""
                ob = hp
                estA, estB = ests
                muls = []
                for h, est, p0 in ((2 * hp, estA, 0), (2 * hp + 1, estB, 64)):
                    pso = psA.tile([P, 640], dtf, tag="psA")
                    for (c0, cw) in NCH:
                        for mt in range(NT):
                            nc.tensor.matmul(
                                pso[:65, c0:c0 + cw],
                                lhsT=v_sb[:, mt, h * 65:h * 65 + 65],
                                rhs=est[:, mt, c0:c0 + cw],
                                start=(mt == 0),
                                stop=(mt == NT - 1),
                            )
                    otr = n2.tile([64, N], dtb, tag="otr")
                    s1 = n1.tile([1, N], dtf, tag="s1")
                    if act_evac:
                        nc.scalar.copy(out=otr[:], in_=pso[:64, :N])
                    else:
                        nc.vector.tensor_copy(out=otr[:], in_=pso[:64, :N])
                    nc.vector.tensor_copy(out=s1[0:1, :], in_=pso[64:65, :N])
                    rec = n1.tile([1, N], dtf, tag="rec")
                    nc.vector.reciprocal_approx_fast(out=rec[0:1, :], in_=s1[0:1, :])
                    rec16 = n1.tile([1, N], dtb, tag="rec16")
                    nc.vector.tensor_copy(out=rec16[0:1, :], in_=rec[0:1, :])
                    recb = n2.tile([64, N], dtb, tag="recb")
                    nc.gpsimd.partition_broadcast(recb[:], rec16[0:1, :])

                    def mk(p0=p0, ob=ob, otr=otr, recb=recb):
                        nc.vector.tensor_mul(
                            out=ot_sb[ob][p0:p0 + 64, :], in0=otr[:], in1=recb[:]
                        )

                    muls.append(mk)
                return muls

            def emit_proj_chunk(ot_sb, b, nt, oc, upto=CT, evac_act=False):
                """Proj chunk; with upto<CT, emits only ct<upto and returns a
                finisher for the remaining cts + evac + store (lets the tail
                start proj before the last head pair's normalize lands).
                evac_act folds the bias in as a K=1 ones-row matmul and
                evacuates on the ACT engine — used near the kernel tail where
                DVE is congested with the last normalize chains."""
                nh = NTS[nt]
                ps = psB.tile([P, 512], dtf, tag="psB")

                def seg(c0, c1):
                    for ct in range(c0, c1):
                        nc.tensor.matmul(
                            ps[:nh],
                            lhsT=ot_sb[ct][:, nt * P:nt * P + nh],
                            rhs=wp_sb[:, ct, oc * 512:(oc + 1) * 512],
                            start=(ct == 0),
                            stop=(ct == CT - 1 and not evac_act),
                        )

                def fin():
                    seg(upto, CT)
                    outt = outpool.tile([P, 512], dtb, tag="out")
                    if evac_act:
                        nc.tensor.matmul(
                            ps[:nh],
                            lhsT=ones1[0:1, :nh],
                            rhs=bpb_sb[0:1, oc * 512:(oc + 1) * 512],
                            start=False,
                            stop=True,
                        )
                        nc.scalar.copy(out=outt[:nh], in_=ps[:nh])
                    else:
                        nc.vector.tensor_add(
                            out=outt[:nh],
                            in0=ps[:nh],
                            in1=bpb_sb[:nh, oc * 512:(oc + 1) * 512],
                        )
                    nc.sync.dma_start(
                        out=y[b, nt * P:nt * P + nh, oc * 512:(oc + 1) * 512],
                        in_=outt[:nh],
                    )

                seg(0, upto)
                if upto == CT:
                    fin()
                    return None
                return fin

            # ---- phase 0: input DMAs in first-needed order, 3 queues.
            # x tiles interleave with wq group-0 tiles so the (x-ct, wq-ct)
            # pairs the first matmul chain consumes arrive together. ----
            nc.sync.dma_start(out=bqk_sb[:], in_=bqk[:])
            x0 = xpool.tile([P, CT, N], dtb, tag="x")
            x0b = xt[0].rearrange("(ct p) n -> p ct n", p=P)
            for ct in range(CT):
                QS[ct % 3].dma_start(out=x0[:, ct], in_=x0b[:, ct])
                QS[(ct + 1) % 3].dma_start(
                    out=wq_sb[:, ct, 0:256], in_=wq[ct * P:(ct + 1) * P, 0:256]
                )
            qi = 0
            for (g0, g1) in [(256, 512), (512, 1024), (1024, 1536),
                             (1536, 2048)]:
                for ct in range(CT):
                    QS[qi % 3].dma_start(
                        out=wq_sb[:, ct, g0:g1],
                        in_=wq[ct * P:(ct + 1) * P, g0:g1],
                    )
                    qi += 1
            nc.sync.dma_start(
                out=bvb_sb[:], in_=bass.AP(tensor=bv, offset=0, ap=[[0, P], [1, C]])
            )
            for ct in range(CT):
                QS[qi % 3].dma_start(
                    out=wq_sb[:, ct, 2 * C:], in_=wq[ct * P:(ct + 1) * P, 2 * C:]
                )
                qi += 1
            for ct in range(CT):
                QS[qi % 3].dma_start(out=wp_sb[:, ct], in_=wp[ct * P:(ct + 1) * P, :])
                qi += 1
            nc.gpsimd.dma_start(
                out=bpb_sb[:], in_=bass.AP(tensor=bpr, offset=0, ap=[[0, P], [1, C]])
            )
            x1 = load_x(1, qoff=2)

            # zero-guards for the PV K=128 pad of m-tile 4: rows 65:128 of the
            # est (mt 4) and v (nt 4) slabs must be exactly 0 in every pool
            # buffer (pool memory persists across reuse; nothing else writes
            # those rows).  Guards run on DVE during the phase-0 DMA window.
            # ---- phase 1: QKV(b0) + V(b0), dense PE stream ----
            qk0 = qkpool.tile([P, 16, N], dtb, tag="qk")
            for ot in range(16):
                emit_qk_tile(x0, qk0, ot, wide=True)
                # zero-guards for the PV K=128 pad of m-tile 4 (rows 65:128 of
                # est mt-4 / v nt-4 slabs stay 0 through pool reuse), spread
                # one per tile through the DVE idle gaps between qk evacs.
                if 8 <= ot <= 11:
                    g = epool.tile([P, NT, N], dtb, tag="est")
                    nc.vector.memset(g[64:128, 4, :], 0.0)
                elif 12 <= ot <= 13:
                    g = vpool.tile([P, NT, H * 65], dtb, tag="v")
                    nc.vector.memset(g[64:128, 4, :], 0.0)
            v0 = alloc_v(0)
            for nt in range(NT):
                for oc in range(2):
                    emit_v_chunk(x0, v0, nt, oc)

            # ---- phases 2+3: attention pipelined one head-pair ahead.
            # step k: S(pair k), fillers, PV(pair k-1), muls(pair k-2).
            # pairs 0..7 = batch 0, pairs 8..15 = batch 1.
            qk1 = qkpool.tile([P, 16, N], dtb, tag="qk")
            v1 = alloc_v(1)
            # per-ct OT tiles: separate dep semaphores per head-pair slab, so
            # proj consumers wait only on the slabs they actually read.
            ot0 = [opool.tile([P, N], dtb, tag="ot", name=f"ot0_{c}") for c in range(CT)]
            ot1 = [opool.tile([P, N], dtb, tag="ot", name=f"ot1_{c}") for c in range(CT)]

            qk_order = [0, 8, 1, 9, 2, 10, 3, 11, 4, 12, 5, 13, 6, 14, 7, 15]
            fillers = [
                (lambda ot=ot: emit_qk_tile(x1, qk1, ot, wide=False))
                for ot in qk_order
            ]
            fillers += [
                (lambda nt=nt: emit_v_chunk(x1, v1, nt, 0)) for nt in range(NT)
            ]
            fillers += [
                (lambda nt=nt: emit_v_chunk(x1, v1, nt, 1)) for nt in range(NT)
            ]
            fillers += [
                (lambda nt=nt, oc=oc, ea=(i >= 7): emit_proj_chunk(
                    ot0, 0, nt, oc, evac_act=ea))
                for i, (oc, nt) in enumerate(
                    (oc, nt) for oc in range(2) for nt in range(NT))
            ]
            # fillers per step (36 over 16 steps + tail step).  Up to two of a
            # step's fillers run INSIDE emit_S (after m-tiles 1/3) so ACT can
            # drain exps and release psA slots before the PE needs them; the
            # rest run between S and PV.  At step 16 the ready muls(14) run
            # first, then fillers cover the last normalize chain of pair 15.
            per = [3, 2, 2, 2, 2, 2, 2, 2, 2, 2, 2, 2, 2, 2, 2, 2, 3]
            fi = 0

            pending_pv = None  # (v_sb, ests, ot_sb, hp)
            pending_muls = []
            for k in range(17):
                step_f = [fillers[fi + j] for j in range(per[k])]
                fi += per[k]
                if k < 16:
                    b = k // 8
                    hp = k % 8
                    ests = emit_S(qk1 if b else qk0, hp, fill=step_f[:2])
                    for f in step_f[2:]:
                        f()
                if pending_pv is not None:
                    muls = emit_PV(*pending_pv, act_evac=(k == 16))
                    if k == 16:
                        for m in pending_muls:
                            m()
                        for f in step_f:
                            f()
                    else:
                        for m in pending_muls:
                            m()
                    pending_muls = muls
                if k < 16:
                    b = k // 8
                    hp = k % 8
                    pending_pv = (v1 if b else v0, ests, ot1 if b else ot0, hp)
            while fi < len(fillers):
                fillers[fi]()
                fi += 1
            for m in pending_muls:
                m()

            # ---- phase 4: proj(b1) tail.  The first two chunks emit ct0..6
            # immediately (independent of the final muls) and finish ct7 after;
            # smallest chunks last for a fast drain. ----
            f0 = emit_proj_chunk(ot1, 1, 0, 0, upto=7, evac_act=True)
            f1 = emit_proj_chunk(ot1, 1, 1, 0, upto=7, evac_act=True)
            f0()
            f1()
            order = [(2, 0), (3, 0), (0, 1), (1, 1), (2, 1), (3, 1), (4, 0), (4, 1)]
            for nt, oc in order:
                emit_proj_chunk(ot1, 1, nt, oc)
    nc.compile()
    return nc


def kernel(x, w_qkv, b_qkv, w_proj, b_proj):
    global LAST_RESULT
    _ensure_ntff_hook()
    from concourse.bass_utils import run_bass_kernel_spmd

    bf16 = ml_dtypes.bfloat16
    x = np.asarray(x, dtype=np.float32)
    w_qkv = np.asarray(w_qkv, dtype=np.float32)
    b_qkv = np.asarray(b_qkv, dtype=np.float32)
    w_proj = np.asarray(w_proj, dtype=np.float32)
    b_proj = np.asarray(b_proj, dtype=np.float32)

    xT = np.ascontiguousarray(np.transpose(x, (0, 2, 1))).astype(bf16)  # [B, C, N]
    wqkvT = np.ascontiguousarray(w_qkv.T).astype(bf16)  # [C, 3C]
    wprojT = np.ascontiguousarray(w_proj.T).astype(bf16)  # [C, C]
    bqk = np.ascontiguousarray(b_qkv[:2 * C].reshape(16, P).T).astype(np.float32)
    bv = np.ascontiguousarray(b_qkv[2 * C:]).astype(bf16)
    bpr = np.ascontiguousarray(b_proj).astype(bf16)

    in_maps = []
    for i in range(NCORES):
        in_maps.append(
            {
                "xt": np.ascontiguousarray(xT[i * BPC:(i + 1) * BPC]),
                "wqkvT": wqkvT,
                "wprojT": wprojT,
                "bqk": bqk,
                "bv": bv,
                "bproj": bpr,
            }
        )

    if "nc" not in _CACHE:
        _CACHE["nc"] = _build_nc()
    nc = _CACHE["nc"]

    res = run_bass_kernel_spmd(nc, in_maps, core_ids=list(range(NCORES)))
    LAST_RESULT = res
    out = np.concatenate([r["y"] for r in res.results], axis=0)
    return np.ascontiguousarray(out.astype(np.float32))


if __name__ == "__main__":
    rng = np.random.default_rng(0)
    x = rng.standard_normal((B, N, C), dtype=np.float32)
    w_qkv = rng.standard_normal((3 * C, C), dtype=np.float32) * C ** -0.5
    b_qkv = rng.standard_normal(3 * C).astype(np.float32) * 0.02
    w_proj = rng.standard_normal((C, C), dtype=np.float32) * C ** -0.5
    b_proj = rng.standard_normal(C).astype(np.float32) * 0.02
    out = kernel(x=x, w_qkv=w_qkv, b_qkv=b_qkv, w_proj=w_proj, b_proj=b_proj)
    print(out.shape, out.dtype)
